# revision 1
# baseline (speedup 1.0000x reference)
"""MoE layer kernel for Trainium2, 8 NeuronCores, data-parallel over tokens.

Problem: x (4, 4096, 1024), router Wr (1024, 8) + br, experts W1 (8,1024,1024)
+ b1, W2 (8,1024,1024) + b2, top-2 softmax routing, dense-equivalent output
out (4, 4096, 1024).

Sharding: 16384 tokens split 8 ways (2048 tokens/core); expert weights
replicated. Math per core (dense over experts, exact vs reference):
  logits = x @ Wr + br ; top2 ; c0 = sigmoid(m1-m2), c1 = 1-c0
  coef_full[t,e] = c0*[e==argmax1] + c1*[e==argmax2]
  out = sum_e coef_full[:,e] * (relu(x @ W1[e] + b1[e]) @ W2[e] + b2[e])
     = sum_e coef_full[:,e] * (relu(...) @ W2[e])  +  coef_full @ b2
Matmuls run in float32r (full PE rate, ~fp22 mantissa).
"""
import sys

sys.path.insert(0, "/opt/trn_rl_repo")

import numpy as np
import concourse.bass as bass
import concourse.mybir as mybir
import concourse.tile as tile
from concourse import bacc
from concourse.bass_utils import run_bass_kernel_spmd
from concourse.masks import make_identity

dt = mybir.dt
AF = mybir.ActivationFunctionType
ALU = mybir.AluOpType

NCORES = 8
B, NOBJ, D = 4, 4096, 1024
H = O = 1024
E = 8
TOK = B * NOBJ          # 16384 tokens total
T = TOK // NCORES       # 2048 tokens per core
TH = T // 2             # half = 1024 tokens (SBUF fits a half)
P = 128

_NC_CACHE = {}


def build_nc(body_reps=1):
    key = ("nc", body_reps)
    if key in _NC_CACHE:
        return _NC_CACHE[key]
    nc = bacc.Bacc("TRN2", target_bir_lowering=False, debug=False)

    xT = nc.dram_tensor("xT", [D, T], dt.float32r, kind="ExternalInput")
    xThi = nc.dram_tensor("xThi", [D, T], dt.float32r, kind="ExternalInput")
    xTlo = nc.dram_tensor("xTlo", [D, T], dt.float32r, kind="ExternalInput")
    wrhi = nc.dram_tensor("wrhi", [D, E], dt.float32r, kind="ExternalInput")
    wrlo = nc.dram_tensor("wrlo", [D, E], dt.float32r, kind="ExternalInput")
    brc = nc.dram_tensor("brc", [E, 1], dt.float32, kind="ExternalInput")
    w1 = nc.dram_tensor("w1", [E, D, H], dt.float32r, kind="ExternalInput")
    b1c = nc.dram_tensor("b1c", [P, E * (H // P)], dt.float32, kind="ExternalInput")
    w2 = nc.dram_tensor("w2", [E, H, O], dt.float32r, kind="ExternalInput")
    b2 = nc.dram_tensor("b2", [E, O], dt.float32r, kind="ExternalInput")
    out = nc.dram_tensor("out", [T, O], dt.float32, kind="ExternalOutput")

    ND = D // P   # 8 d-slices
    NH = H // P   # 8 h-slices
    NT = TH // P  # 8 token tiles per half
    NC2 = TH // 512  # 2 token chunks of 512 per half
    NOC = O // 512   # 2 o chunks

    with tile.TileContext(nc) as tc:
        with (
            tc.tile_pool(name="const", bufs=1) as cpool,
            tc.tile_pool(name="xt", bufs=ND + 1) as xt_pool,
            tc.tile_pool(name="w1p", bufs=6) as w1_pool,
            tc.tile_pool(name="w2p", bufs=NH + 2) as w2_pool,
            tc.tile_pool(name="hp", bufs=NH + 1) as h_pool,
            tc.tile_pool(name="acc", bufs=NT) as acc_pool,
            tc.tile_pool(name="rt", bufs=2) as rt_pool,
            tc.tile_pool(name="cfp", bufs=NT + 1) as cf_pool,
            tc.tile_pool(name="ps1", bufs=4, space="PSUM") as ps1,
            tc.tile_pool(name="ps2", bufs=2, space="PSUM") as ps2,
            tc.tile_pool(name="psm", bufs=1, space="PSUM") as psm,
        ):
            ident = cpool.tile([P, P], dt.float32)
            make_identity(nc, ident[:])
            # hi/lo split of router inputs is done on host: hi parts are
            # m11-exact so the PE's fp32r read rounding is a no-op and the
            # 4 accumulated hi/lo products give ~fp32-exact logits
            wr_hi = cpool.tile([P, ND * E], dt.float32r)
            wr_lo = cpool.tile([P, ND * E], dt.float32r)
            for ds in range(ND):
                nc.sync.dma_start(wr_hi[:, ds * E:(ds + 1) * E], wrhi[ds * P:(ds + 1) * P, :])
                nc.sync.dma_start(wr_lo[:, ds * E:(ds + 1) * E], wrlo[ds * P:(ds + 1) * P, :])
            brc_sb = cpool.tile([E, 1], dt.float32)
            nc.sync.dma_start(brc_sb[:], brc[:])
            b1c_sb = cpool.tile([P, E * NH], dt.float32)
            nc.sync.dma_start(b1c_sb[:], b1c[:])
            b2_sb = cpool.tile([E, O], dt.float32r)
            nc.sync.dma_start(b2_sb[:], b2[:])

            for rep in range(body_reps):
              for half in range(2):
                t0 = half * TH
                # ---- X^T tiles for this half: 8 x (128, 1024), float32r
                xt = []
                for ds in range(ND):
                    xti = xt_pool.tile([P, TH], dt.float32r, tag="xt")
                    nc.gpsimd.dma_start(xti[:], xT[ds * P:(ds + 1) * P, t0:t0 + TH])
                    xt.append(xti)

                # ---- router: logitsT (8, TH) then transpose to token-major
                logitsT = rt_pool.tile([E, TH], dt.float32, tag="logitsT")
                for c in range(NC2):
                    cs = slice(c * 512, (c + 1) * 512)
                    pr = psm.tile([E, 512], dt.float32, tag="psr")
                    for ds in range(ND):
                        xhi = rt_pool.tile([P, 512], dt.float32r, tag="xhi")
                        xlo = rt_pool.tile([P, 512], dt.float32r, tag="xlo")
                        nc.sync.dma_start(xhi[:], xThi[ds * P:(ds + 1) * P, t0 + c * 512:t0 + (c + 1) * 512])
                        nc.sync.dma_start(xlo[:], xTlo[ds * P:(ds + 1) * P, t0 + c * 512:t0 + (c + 1) * 512])
                        whi_s = wr_hi[:, ds * E:(ds + 1) * E]
                        wlo_s = wr_lo[:, ds * E:(ds + 1) * E]
                        for mi, (wop, xop) in enumerate(
                                [(whi_s, xhi), (wlo_s, xhi), (whi_s, xlo), (wlo_s, xlo)]):
                            nc.tensor.matmul(
                                out=pr[:], lhsT=wop, rhs=xop[:],
                                start=(ds == 0 and mi == 0),
                                stop=(ds == ND - 1 and mi == 3),
                            )
                    nc.vector.tensor_scalar(logitsT[:, cs], pr[:], brc_sb[:, 0:1], None, op0=ALU.add)

                coef = []    # token-major coef_full tiles (128, 8) fp32
                coefT = rt_pool.tile([E, TH], dt.float32r, tag="coefT")
                for tt in range(NT):
                    ts_ = slice(tt * P, (tt + 1) * P)
                    pl = psm.tile([P, E], dt.float32, tag="pst")
                    nc.tensor.transpose(out=pl[:], in_=logitsT[:, ts_], identity=ident[:E, :E])
                    lg = rt_pool.tile([P, E], dt.float32, tag="lg")
                    nc.scalar.copy(lg[:], pl[:])
                    top = rt_pool.tile([P, 8], dt.float32, tag="top")
                    topi = rt_pool.tile([P, 8], dt.uint32, tag="topi")
                    nc.vector.max_with_indices(top[:], topi[:], lg[:])
                    m1, m2 = top[:, 0:1], top[:, 1:2]
                    d01 = rt_pool.tile([P, 1], dt.float32, tag="d01")
                    nc.vector.tensor_sub(d01[:], m1, m2)
                    c0 = rt_pool.tile([P, 1], dt.float32, tag="c0")
                    nc.scalar.activation(out=c0[:], in_=d01[:], func=AF.Sigmoid)
                    c1 = rt_pool.tile([P, 1], dt.float32, tag="c1")
                    nc.vector.tensor_scalar(c1[:], c0[:], -1.0, 1.0, op0=ALU.mult, op1=ALU.add)
                    eq0 = rt_pool.tile([P, E], dt.float32, tag="eq0")
                    nc.vector.tensor_scalar(eq0[:], lg[:], m1, None, op0=ALU.is_equal)
                    eq1 = rt_pool.tile([P, E], dt.float32, tag="eq1")
                    nc.vector.tensor_scalar(eq1[:], lg[:], m2, None, op0=ALU.is_equal)
                    cf = cf_pool.tile([P, E], dt.float32, tag="cf")
                    nc.vector.tensor_scalar(cf[:], eq0[:], c0[:], None, op0=ALU.mult)
                    nc.vector.scalar_tensor_tensor(
                        out=cf[:], in0=eq1[:], scalar=c1[:], in1=cf[:],
                        op0=ALU.mult, op1=ALU.add,
                    )
                    coef.append(cf)
                    # transpose coef tile -> coefT columns (cast to f32r via copy)
                    pc = psm.tile([E, P], dt.float32, tag="pst")
                    nc.tensor.transpose(out=pc[:], in_=cf[:], identity=ident[:])
                    nc.vector.tensor_copy(coefT[:, ts_], pc[:])

                # ---- init outacc with coef_full @ b2  (K=8 matmul)
                outacc = []
                for tt in range(NT):
                    ts_ = slice(tt * P, (tt + 1) * P)
                    oa = acc_pool.tile([P, O], dt.float32, tag="acc")
                    for oc in range(NOC):
                        os_ = slice(oc * 512, (oc + 1) * 512)
                        pb = ps2.tile([P, 512], dt.float32, tag="ps2")
                        nc.tensor.matmul(out=pb[:], lhsT=coefT[:, ts_], rhs=b2_sb[:, os_],
                                         start=True, stop=True)
                        nc.scalar.copy(oa[:, os_], pb[:])
                    outacc.append(oa)

                # ---- experts
                for e in range(E):
                    # mm1: H^T = relu(W1[e]^T x^T + b1)  in h-groups of 4 slices
                    hbuf = []
                    for c in range(NC2):
                        cs = slice(c * 512, (c + 1) * 512)
                        for grp in range(2):
                            hs0 = grp * 4
                            pgrp = [ps1.tile([P, 512], dt.float32, tag="ps1", name=f"ps1_{hi}")
                                    for hi in range(4)]
                            for ds in range(ND):
                                w1t = w1_pool.tile([P, 512], dt.float32r, tag="w1")
                                nc.sync.dma_start(
                                    w1t[:], w1[e, ds * P:(ds + 1) * P, hs0 * P:(hs0 + 4) * P])
                                for hi in range(4):
                                    nc.tensor.matmul(
                                        out=pgrp[hi][:],
                                        lhsT=w1t[:, hi * P:(hi + 1) * P],
                                        rhs=xt[ds][:, cs],
                                        start=(ds == 0), stop=(ds == ND - 1),
                                    )
                            for hi in range(4):
                                hs = hs0 + hi
                                if c == 0:
                                    ht = h_pool.tile([P, TH], dt.float32r, tag="h")
                                    hbuf.append(ht)
                                nc.scalar.activation(
                                    out=hbuf[hs][:, cs], in_=pgrp[hi][:], func=AF.Relu,
                                    bias=b1c_sb[:, e * NH + hs:e * NH + hs + 1],
                                )
                    # reorder hbuf: created in order hs = 0,1,2,3 (c=0 grp0), 4..7
                    # mm2: out += coef_e * (H^T)^T W2[e]
                    for oc in range(NOC):
                        os_ = slice(oc * 512, (oc + 1) * 512)
                        w2ts = []
                        for hs in range(NH):
                            w2t = w2_pool.tile([P, 512], dt.float32r, tag="w2")
                            nc.sync.dma_start(
                                w2t[:], w2[e, hs * P:(hs + 1) * P, os_])
                            w2ts.append(w2t)
                        for tt in range(NT):
                            ts_ = slice(tt * P, (tt + 1) * P)
                            py = ps2.tile([P, 512], dt.float32, tag="ps2")
                            for hs in range(NH):
                                nc.tensor.matmul(
                                    out=py[:], lhsT=hbuf[hs][:, ts_], rhs=w2ts[hs][:],
                                    start=(hs == 0), stop=(hs == NH - 1),
                                )
                            nc.vector.scalar_tensor_tensor(
                                out=outacc[tt][:, os_], in0=py[:],
                                scalar=coef[tt][:, e:e + 1], in1=outacc[tt][:, os_],
                                op0=ALU.mult, op1=ALU.add,
                            )

                for tt in range(NT):
                    nc.sync.dma_start(out[t0 + tt * P:t0 + (tt + 1) * P, :], outacc[tt][:])

    nc.compile()
    _NC_CACHE[key] = nc
    return nc


def prep_in_maps(x, Wr, br, W1, b1, W2, b2):
    x = np.ascontiguousarray(np.asarray(x, dtype=np.float32))
    Wr = np.ascontiguousarray(np.asarray(Wr, dtype=np.float32))
    br = np.asarray(br, dtype=np.float32)
    W1 = np.ascontiguousarray(np.asarray(W1, dtype=np.float32))
    b1 = np.asarray(b1, dtype=np.float32)
    W2 = np.ascontiguousarray(np.asarray(W2, dtype=np.float32))
    b2 = np.ascontiguousarray(np.asarray(b2, dtype=np.float32))
    xf = x.reshape(TOK, D)
    b1c = np.ascontiguousarray(b1.reshape(E, H // P, P).transpose(2, 0, 1).reshape(P, E * (H // P)))
    brc = np.ascontiguousarray(br.reshape(E, 1))
    MASK11 = np.uint32(0xFFFFF000)
    xhi = (xf.view(np.uint32) & MASK11).view(np.float32)
    xlo = xf - xhi
    wrhi = (Wr.view(np.uint32) & MASK11).view(np.float32)
    wrlo = Wr - wrhi
    in_maps = []
    for c in range(NCORES):
        sl = slice(c * T, (c + 1) * T)
        in_maps.append({
            "xT": np.ascontiguousarray(xf[sl].T),
            "xThi": np.ascontiguousarray(xhi[sl].T),
            "xTlo": np.ascontiguousarray(xlo[sl].T),
            "wrhi": wrhi, "wrlo": wrlo,
            "brc": brc, "w1": W1, "b1c": b1c, "w2": W2, "b2": b2,
        })
    return in_maps


def kernel(x, Wr, br, W1, b1, W2, b2):
    x = np.ascontiguousarray(np.asarray(x, dtype=np.float32))
    Wr = np.ascontiguousarray(np.asarray(Wr, dtype=np.float32))
    br = np.asarray(br, dtype=np.float32)
    W1 = np.ascontiguousarray(np.asarray(W1, dtype=np.float32))
    b1 = np.asarray(b1, dtype=np.float32)
    W2 = np.ascontiguousarray(np.asarray(W2, dtype=np.float32))
    b2 = np.ascontiguousarray(np.asarray(b2, dtype=np.float32))

    xf = x.reshape(TOK, D)
    b1c = np.ascontiguousarray(b1.reshape(E, H // P, P).transpose(2, 0, 1).reshape(P, E * (H // P)))
    brc = np.ascontiguousarray(br.reshape(E, 1))

    MASK11 = np.uint32(0xFFFFF000)
    xhi = (xf.view(np.uint32) & MASK11).view(np.float32)
    xlo = xf - xhi
    wrhi = (Wr.view(np.uint32) & MASK11).view(np.float32)
    wrlo = Wr - wrhi

    nc = build_nc()
    in_maps = []
    for c in range(NCORES):
        sl = slice(c * T, (c + 1) * T)
        in_maps.append({
            "xT": np.ascontiguousarray(xf[sl].T),
            "xThi": np.ascontiguousarray(xhi[sl].T),
            "xTlo": np.ascontiguousarray(xlo[sl].T),
            "wrhi": wrhi, "wrlo": wrlo,
            "brc": brc, "w1": W1, "b1c": b1c, "w2": W2, "b2": b2,
        })
    res = run_bass_kernel_spmd(nc, in_maps, core_ids=list(range(NCORES)))
    out = np.concatenate([res.results[c]["out"] for c in range(NCORES)], axis=0)
    return out.reshape(B, NOBJ, O)



# revision 28
# speedup vs baseline: 2.2435x; 2.2435x over previous
"""Sparse top-2 MoE kernel for Trainium2, 8 NeuronCores, data-parallel tokens.

Problem: x (4, 4096, 1024), router Wr (1024, 8) + br, experts W1 (8,1024,1024)
+ b1, W2 (8,1024,1024) + b2, top-2 softmax routing, out (4, 4096, 1024).

Strategy (vs dense baseline that computed all 8 experts per token):
- 16384 tokens split 8 ways (2048/core) with a host-side balanced shuffle so
  per-(core,expert) routed counts stay near the mean; expert weights
  replicated in fp16.
- Router logits computed fp32-exact via fp16 hi/lo split (x = x16 + 2^-11*xr,
  Wr likewise), so top-2 selection matches the fp32 reference bit-for-bit
  (min top2/top3 logit gap 6.5e-6 >> 2e-6 reconstruction error).
- On-device compaction: per expert, sparse_gather compacts the selected
  token ids + gate coefs out of a wrapped-16 mask layout; static per-expert
  capacities (actual count + 16 margin, rounded to 128) keep the NEFF static.
- dma_gather (transposed) fetches only the routed tokens' x rows in fp16;
  mm1/mm2 run fp16 at full PE rate on ~2*T tokens instead of 8*T.
- Combine: y tiles scaled by gate coef on evac, dma_scatter_add'ed into the
  bf16 output accumulator (pre-initialized with the coef@b2 bias term).
Host adds nothing: out = per-core ysum rows un-permuted, upcast to f32.
"""
import sys

sys.path.insert(0, "/opt/trn_rl_repo")

import numpy as np
import ml_dtypes
import concourse.bass as bass
import concourse.mybir as mybir
import concourse.tile as tile
from concourse import bacc, library_config
from concourse.bass_utils import run_bass_kernel_spmd
from concourse.masks import make_identity

dt = mybir.dt
AF = mybir.ActivationFunctionType
ALU = mybir.AluOpType

NCORES = 8
B, NOBJ, D = 4, 4096, 1024
H = O = 1024
E = 8
TOK = B * NOBJ          # 16384 tokens total
T = TOK // NCORES       # 2048 tokens per core
P = 128
ND = D // P             # 8 d-slices
NH = H // P             # 8 h-slices
NTT = T // P            # 16 token tiles
RS = 2048.0             # hi/lo residual scale 2^11

_NC_CACHE = {}


def build_nc(caps, body_reps=1, dbg=False):
    """caps: tuple of 8 per-expert capacities (multiples of 128)."""
    key = (tuple(caps), body_reps, dbg)
    if key in _NC_CACHE:
        return _NC_CACHE[key]
    nc = bacc.Bacc("TRN2", target_bir_lowering=False, debug=False)

    xf16T = nc.dram_tensor("xf16T", [D, T], dt.float16, kind="ExternalInput")
    xresT = nc.dram_tensor("xresT", [D, T], dt.float16, kind="ExternalInput")
    xrow = nc.dram_tensor("xrow", [T, D], dt.float16, kind="ExternalInput")
    wrcat = nc.dram_tensor("wrcat", [D, 16], dt.float16, kind="ExternalInput")
    brc = nc.dram_tensor("brc", [E, 1], dt.float32, kind="ExternalInput")
    w1 = nc.dram_tensor("w1", [E, D, H], dt.float16, kind="ExternalInput")
    b1c = nc.dram_tensor("b1c", [P, E * NH], dt.float32, kind="ExternalInput")
    w2 = nc.dram_tensor("w2", [E, H, O], dt.float16, kind="ExternalInput")
    b2 = nc.dram_tensor("b2", [E, O], dt.float32, kind="ExternalInput")
    iop1 = nc.dram_tensor("iop1", [16, E * P], dt.float32, kind="ExternalInput")
    wiot = nc.dram_tensor("wiot", [16, 40], dt.float32, kind="ExternalInput")
    scrm = nc.dram_tensor("scrm", [3 * T], dt.float32, kind="Internal")
    lgscr = nc.dram_tensor("lgscr", [E * T], dt.float32, kind="Internal")
    cfscr = nc.dram_tensor("cfscr", [E * T], dt.float32, kind="Internal")
    gscr = nc.dram_tensor("gscr", [E * 640], dt.float32, kind="Internal")
    nfscr = nc.dram_tensor("nfscr", [E * 16], dt.float32, kind="Internal")
    ysum = nc.dram_tensor("ysum", [T, O], dt.bfloat16, kind="ExternalOutput")
    if dbg:
        CMX = max(caps)
        dlog = nc.dram_tensor("dlog", [E, T], dt.float32, kind="ExternalOutput")
        dsti = nc.dram_tensor("dsti", [16, E * P], dt.float32, kind="ExternalOutput")
        dsci = nc.dram_tensor("dsci", [16, E * P], dt.float32, kind="ExternalOutput")
        dtok = nc.dram_tensor("dtok", [16, E * (CMX // 16)], dt.float32, kind="ExternalOutput")
        dcfw = nc.dram_tensor("dcfw", [16, E * (CMX // 16)], dt.float32, kind="ExternalOutput")
        dgat = nc.dram_tensor("dgat", [P, E * (CMX // P)], dt.float32, kind="ExternalOutput")
        didx = nc.dram_tensor("didx", [P, E * (CMX // 16)], dt.int16, kind="ExternalOutput")

    CT = [c // 16 for c in caps]    # wrapped free width per expert
    TC = [c // P for c in caps]     # token tiles per expert

    with tile.TileContext(nc) as tc:
        with (
            tc.tile_pool(name="const", bufs=1) as cpool,
            tc.tile_pool(name="rx", bufs=6) as rxp,
            tc.tile_pool(name="lgt", bufs=1) as lgp,
            tc.tile_pool(name="topk", bufs=2) as tkp,
            tc.tile_pool(name="mask", bufs=1) as mkp,
            tc.tile_pool(name="sg", bufs=1) as sgp,
            tc.tile_pool(name="idx", bufs=3) as idxp,
            tc.tile_pool(name="wt", bufs=2) as wtp,
            tc.tile_pool(name="xg", bufs=2) as xgp,
            tc.tile_pool(name="hb", bufs=2) as hbp,
            tc.tile_pool(name="yb", bufs=3) as ybp,
            tc.tile_pool(name="pr", bufs=2, space="PSUM") as ppr,
            tc.tile_pool(name="p1", bufs=2, space="PSUM") as pp1,
            tc.tile_pool(name="p2", bufs=2, space="PSUM") as pp2,
        ):
            nc.gpsimd.load_library(library_config.sparse_gather)

            ident = cpool.tile([P, P], dt.float32)
            wrsb = cpool.tile([P, ND * 16], dt.float16)
            nc.sync.dma_start(
                wrsb[:], bass.AP(tensor=wrcat[:, :].tensor, offset=0,
                                 ap=[[16, P], [P * 16, ND], [1, 16]]))
            brsb = cpool.tile([E, 1], dt.float32)
            b1sb = cpool.tile([P, E * NH], dt.float32)
            b2sb = cpool.tile([E, O], dt.float32r)
            iosb = cpool.tile([16, E * P], dt.float32)
            wiosb = cpool.tile([16, 40], dt.float32)

            for rep in range(body_reps):
                # ---- router: logitsT [8, 2048] fp32-exact via hi/lo fp16
                # (x-tile DMAs emitted first so the PE isn't gated behind
                # the weight prefetch on the SP queue)
                logitsT = lgp.tile([E, T], dt.float32, tag="lgt")
                xtiles = []
                for ch in range(4):
                    cs = slice(ch * 512, (ch + 1) * 512)
                    row = []
                    for ds in range(ND):
                        x16 = rxp.tile([P, 512], dt.float16, tag="x16")
                        xrs = rxp.tile([P, 512], dt.float16, tag="xrs")
                        qe = nc.sync if ds % 2 == 0 else nc.scalar
                        qe.dma_start(x16[:], xf16T[ds * P:(ds + 1) * P, cs])
                        qe.dma_start(xrs[:], xresT[ds * P:(ds + 1) * P, cs])
                        row.append((x16, xrs))
                    xtiles.append(row)

                if rep == 0:
                    nc.sync.dma_start(brsb[:], brc[:])
                    make_identity(nc, ident[:])
                    nc.scalar.dma_start(b1sb[:], b1c[:])
                    nc.scalar.dma_start(b2sb[:], b2[:].bitcast(dt.float32r))
                    nc.scalar.dma_start(iosb[:], iop1[:])
                    nc.scalar.dma_start(wiosb[:], wiot[:])
                # ---- w1/w2 prefetch for expert 0 (off the SP queue)
                w1sb = wtp.tile([P, ND, H], dt.float16, tag="w1")
                w2sb = wtp.tile([P, NH, O], dt.float16, tag="w2")
                for ds in range(ND):
                    nc.gpsimd.dma_start(
                        w1sb[:, ds, :],
                        bass.AP(tensor=w1[:, :, :].tensor, offset=ds * P * H,
                                ap=[[H, P], [1, H]]))
                    nc.scalar.dma_start(
                        w2sb[:, ds, :],
                        bass.AP(tensor=w2[:, :, :].tensor, offset=ds * P * O,
                                ap=[[O, P], [1, O]]))

                for ch in range(4):
                    cs = slice(ch * 512, (ch + 1) * 512)
                    pA = ppr.tile([E, 512], dt.float32, tag="pA")
                    pC = ppr.tile([E, 512], dt.float32, tag="pC", bufs=1)
                    for ds in range(ND):
                        x16, xrs = xtiles[ch][ds]
                        nc.tensor.matmul(
                            out=pA[:], lhsT=wrsb[:, 16 * ds:16 * ds + 8],
                            rhs=x16[:], start=(ds == 0), stop=(ds == ND - 1))
                        nc.tensor.matmul(
                            out=pC[:], lhsT=wrsb[:, 16 * ds + 8:16 * (ds + 1)],
                            rhs=x16[:], start=(ds == 0), stop=False)
                        nc.tensor.matmul(
                            out=pC[:], lhsT=wrsb[:, 16 * ds:16 * ds + 8],
                            rhs=xrs[:], start=False, stop=(ds == ND - 1))
                    t1 = tkp.tile([E, 512], dt.float32, tag="t1")
                    nc.vector.tensor_scalar(
                        t1[:], pC[:], 1.0 / RS, None, op0=ALU.mult)
                    nc.vector.scalar_tensor_tensor(
                        out=logitsT[:, cs], in0=pA[:], scalar=brsb[:, 0:1],
                        in1=t1[:], op0=ALU.add, op1=ALU.add)
                    nc.sync.dma_start(
                        bass.AP(tensor=lgscr[:].tensor, offset=ch * 512,
                                ap=[[T, E], [1, 512]]),
                        logitsT[:, cs])

                # ---- top2, grouped: one transposed [128, 16*8] block
                m12c = tkp.tile([P, 48], dt.float32, tag="m12c")
                plA = pp1.tile([P, NTT * E], dt.float32, tag="pst", bufs=1)
                for tt in range(NTT):
                    nc.tensor.matmul(
                        out=plA[:, E * tt:E * (tt + 1)],
                        lhsT=logitsT[:, tt * P:(tt + 1) * P],
                        rhs=ident[:E, :E], is_transpose=True,
                        start=(tt == 0), stop=(tt == NTT - 1),
                        skip_group_check=True)
                lgall = tkp.tile([P, NTT * E], dt.float32, tag="lgall")
                nc.scalar.copy(lgall[:], plA[:])
                lg3 = lgall[:].rearrange("p (t e) -> p t e", e=E)
                nc.vector.tensor_reduce(m12c[:, 0:16], lg3,
                                        axis=mybir.AxisListType.X,
                                        op=ALU.max)
                eqm = tkp.tile([P, NTT * E], dt.float32, tag="eqm")
                eq3 = eqm[:].rearrange("p (t e) -> p t e", e=E)
                nc.vector.scalar_tensor_tensor(
                    out=eq3, in0=lg3, scalar=0.0,
                    in1=m12c[:, 0:16].unsqueeze(2).broadcast_to([P, NTT, E]),
                    op0=ALU.bypass, op1=ALU.is_equal)
                nc.vector.scalar_tensor_tensor(
                    out=eq3, in0=eq3, scalar=-1e30, in1=lg3,
                    op0=ALU.mult, op1=ALU.add)
                nc.vector.tensor_reduce(m12c[:, 16:32], eq3,
                                        axis=mybir.AxisListType.X,
                                        op=ALU.max)
                d01a = tkp.tile([P, 16], dt.float32, tag="d01a")
                nc.vector.scalar_tensor_tensor(
                    out=d01a[:], in0=m12c[:, 16:32], scalar=-1.0,
                    in1=m12c[:, 0:16], op0=ALU.mult, op1=ALU.add)
                nc.scalar.activation(out=m12c[:, 32:48], in_=d01a[:],
                                     func=AF.Sigmoid)

                if dbg:
                    nc.sync.dma_start(dlog[:], logitsT[:])
                # ---- to wrapped-16 layout via DRAM roundtrip
                nc.scalar.dma_start(
                    bass.AP(tensor=scrm[:].tensor, offset=0,
                            ap=[[1, P], [T, 3], [P, 16]]),
                    m12c[:])
                # layout [16, E*128]: expert e in free cols [128e, 128e+128),
                # wrapped-16 within (gpsimd ISA ops need partition base 0).
                # m1/m2/c0 stay [16,128] and broadcast along the expert dim.
                mt = [mkp.tile([16, E * P], dt.float32, name=f"mt{i}")
                      for i in range(5)]
                lgW, eq1, eq2, cfE, tmpE = mt
                m1S = mkp.tile([16, P], dt.float32, name="m1S")
                m2S = mkp.tile([16, P], dt.float32, name="m2S")
                c0S = mkp.tile([16, P], dt.float32, name="c0S")
                c1S = mkp.tile([16, P], dt.float32, name="c1S")
                for g in range(8):
                    gs = slice(P * g, P * (g + 1))
                    nc.sync.dma_start(
                        lgW[:, gs],
                        bass.AP(tensor=lgscr[:].tensor, offset=g * T,
                                ap=[[1, 16], [16, P]]))
                nc.scalar.dma_start(
                    m1S[:], bass.AP(tensor=scrm[:].tensor, offset=0,
                                    ap=[[1, 16], [16, P]]))
                nc.scalar.dma_start(
                    m2S[:], bass.AP(tensor=scrm[:].tensor, offset=T,
                                    ap=[[1, 16], [16, P]]))
                nc.scalar.dma_start(
                    c0S[:], bass.AP(tensor=scrm[:].tensor, offset=2 * T,
                                    ap=[[1, 16], [16, P]]))
                nc.vector.tensor_scalar(c1S[:], c0S[:], -1.0, 1.0,
                                        op0=ALU.mult, op1=ALU.add)


                # ---- masks and compaction inputs  [128,128]
                def tt_op(out, a, b, op):
                    nc.vector.scalar_tensor_tensor(
                        out=out, in0=a, scalar=0.0, in1=b,
                        op0=ALU.bypass, op1=op)

                # per-expert mask slices -> compaction, e0 first so the
                # expert pipeline starts while later experts still compact
                CM = max(caps)
                CM16 = CM // 16
                toks, cfs, nfs, idxrs, gatTs = [], [], [], [], []
                for e in range(E):
                    es = slice(P * e, P * (e + 1))
                    nc.vector.scalar_tensor_tensor(
                        out=eq1[:, es], in0=lgW[:, es], scalar=0.0,
                        in1=m1S[:], op0=ALU.bypass, op1=ALU.is_equal)
                    nc.vector.scalar_tensor_tensor(
                        out=eq2[:, es], in0=lgW[:, es], scalar=0.0,
                        in1=m2S[:], op0=ALU.bypass, op1=ALU.is_equal)
                    # cfE = eq1*c0 + eq2*c1 (coef_full, kept for b2 pass)
                    nc.vector.scalar_tensor_tensor(
                        out=cfE[:, es], in0=eq1[:, es], scalar=0.0,
                        in1=c0S[:], op0=ALU.bypass, op1=ALU.mult)
                    nc.vector.scalar_tensor_tensor(
                        out=tmpE[:, es], in0=eq2[:, es], scalar=0.0,
                        in1=c1S[:], op0=ALU.bypass, op1=ALU.mult)
                    tt_op(cfE[:, es], cfE[:, es], tmpE[:, es], ALU.add)
                    # selm2 = 2*(eq1+eq2) - 1
                    tt_op(eq1[:, es], eq1[:, es], eq2[:, es], ALU.add)
                    nc.vector.tensor_scalar(eq1[:, es], eq1[:, es], 2.0,
                                            -1.0, op0=ALU.mult, op1=ALU.add)
                    # sci = (cfE+1)*selm2 ; sti = (tau+1)*selm2
                    nc.vector.scalar_tensor_tensor(
                        out=tmpE[:, es], in0=cfE[:, es], scalar=1.0,
                        in1=eq1[:, es], op0=ALU.add, op1=ALU.mult)
                    tt_op(eq2[:, es], iosb[:, es], eq1[:, es], ALU.mult)
                    # compact this expert now (Pool runs these in order)
                    tokw = sgp.tile([16, CT[e]], dt.float32, name=f"tok{e}")
                    cfw = sgp.tile([16, CT[e]], dt.float32, name=f"cf{e}")
                    nf1 = sgp.tile([1, 1], dt.uint32, name=f"nf1{e}")
                    nf2 = sgp.tile([1, 1], dt.uint32, name=f"nf2{e}")
                    nc.gpsimd.sparse_gather(tokw[:], eq2[:, es],
                                            num_found=nf1[:])
                    nc.gpsimd.sparse_gather(cfw[:], tmpE[:, es],
                                            num_found=nf2[:])
                    toks.append(tokw)
                    cfs.append(cfw)
                    nfs.append(nf1)
                    # spill coef_full column block for the b2 pass
                    nc.sync.dma_start(
                        bass.AP(tensor=cfscr[:].tensor, offset=e * T,
                                ap=[[1, 16], [16, P]]),
                        cfE[:, P * e:P * (e + 1)])
                    # nf -> [16,1] via free-broadcast + DRAM fold (no gpsimd)
                    nfw = idxp.tile([1, 16], dt.float32, tag="nfw")
                    nc.vector.tensor_copy(
                        nfw[:], nfs[e][:].broadcast_to([1, 16]))
                    nc.scalar.dma_start(
                        bass.AP(tensor=nfscr[:].tensor, offset=e * 16,
                                ap=[[1, 1], [1, 16]]),
                        nfw[:])
                    nfT = idxp.tile([16, 1], dt.float32, tag="nfT")
                    nc.scalar.dma_start(
                        nfT[:], bass.AP(tensor=nfscr[:].tensor,
                                        offset=e * 16, ap=[[1, 16], [1, 1]]))
                    tmask = idxp.tile([16, CT[e]], dt.float32, tag="tmask")
                    nc.vector.tensor_scalar(tmask[:], wiosb[:, :CT[e]],
                                            nfT[:, 0:1], None, op0=ALU.is_lt)
                    # idx list: tau+1 -> tau, tail -> 0, int16, replicate
                    tokc = idxp.tile([16, CM16], dt.float32, tag="tokc")
                    nc.vector.memset(tokc[:], 0.0)
                    nc.vector.tensor_scalar(tokc[:, :CT[e]], tokw[:],
                                            -1.0, 0.0, op0=ALU.add,
                                            op1=ALU.max)
                    tt_op(tokc[:, :CT[e]], tokc[:, :CT[e]], tmask[:],
                          ALU.mult)
                    idxr = sgp.tile([P, CM16], dt.int16, name=f"idxr{e}")
                    nc.vector.tensor_copy(idxr[0:16, :], tokc[:])
                    nc.sync.dma_start(idxr[16:32, :], idxr[0:16, :])
                    nc.sync.dma_start(idxr[32:64, :], idxr[0:32, :])
                    nc.sync.dma_start(idxr[64:128, :], idxr[0:64, :])
                    idxrs.append(idxr)
                    # gating (coef+1 -> coef, 0 in padded tail)
                    gatw = idxp.tile([16, CM16], dt.float32, tag="gatw")
                    nc.vector.memset(gatw[:], 0.0)
                    nc.vector.tensor_scalar(gatw[:, :CT[e]], cfw[:],
                                            -1.0, 0.0, op0=ALU.add,
                                            op1=ALU.max)
                    tt_op(gatw[:, :CT[e]], gatw[:, :CT[e]], tmask[:],
                          ALU.mult)
                    nc.scalar.dma_start(
                        bass.AP(tensor=gscr[:].tensor, offset=e * 640,
                                ap=[[1, 16], [16, CT[e]]]),
                        gatw[:, :CT[e]])
                    gatT = sgp.tile([P, CM // P], dt.float32,
                                    name=f"gatT{e}")
                    nc.scalar.dma_start(
                        gatT[:, :TC[e]],
                        bass.AP(tensor=gscr[:].tensor, offset=e * 640,
                                ap=[[1, P], [P, TC[e]]]))
                    gatTs.append(gatT)
                sti = eq2
                sci = tmpE
                if dbg:
                    nc.sync.dma_start(dsti[:], sti[:])
                    nc.sync.dma_start(dsci[:], sci[:])
                coefT = lgp.tile([E, T], dt.float32r, tag="lgt")
                nc.scalar.dma_start(coefT[:], cfscr[:].bitcast(dt.float32r))
                if dbg:
                    for e in range(E):
                        nc.sync.dma_start(
                            dtok[:, e * (CMX // 16):e * (CMX // 16) + CT[e]],
                            toks[e][:])
                        nc.sync.dma_start(
                            dcfw[:, e * (CMX // 16):e * (CMX // 16) + CT[e]],
                            cfs[e][:])
                nc.gpsimd.load_library(library_config.mlp)

                # ---- b2 bias pass: ysum = coefT.T @ b2 (bf16 init)
                for tt in range(NTT):
                    ts_ = slice(tt * P, (tt + 1) * P)
                    yb = ybp.tile([P, 1, O], dt.bfloat16, tag="ybias")
                    for oc in range(2):
                        os_ = slice(oc * 512, (oc + 1) * 512)
                        pb = pp2.tile([P, 512], dt.float32, tag="p2")
                        nc.tensor.matmul(out=pb[:], lhsT=coefT[:, ts_],
                                         rhs=b2sb[:, os_], start=True,
                                         stop=True)
                        nc.vector.tensor_copy(yb[:, 0, os_], pb[:])
                    nc.scalar.dma_start(ysum[ts_, :], yb[:, 0, :])

                # ---- experts
                for e in range(E):
                    C = caps[e]
                    idxr = idxrs[e]
                    gatT = gatTs[e]
                    if dbg:
                        nc.sync.dma_start(
                            dgat[:, e * (CMX // P):e * (CMX // P) + TC[e]],
                            gatT[:, :TC[e]])
                        nc.sync.dma_start(
                            didx[:, e * (CMX // 16):e * (CMX // 16) + CT[e]],
                            idxr[:, :CT[e]])
                    # dispatch: gather routed tokens' x rows (transposed)
                    xg = xgp.tile([P, ND, CM], dt.float16, tag="xg")
                    nc.gpsimd.dma_gather(xg[:], xrow[:, :], idxr[:], CM, CM,
                                         D, transpose=True)

                    # mm1: h = relu(x @ W1[e] + b1[e]), fp16
                    hsb = hbp.tile([P, NH, CM], dt.float16, tag="h")
                    nch = (C + 511) // 512
                    for chk in range(nch):
                        cw = min(512, C - chk * 512)
                        ks = slice(chk * 512, chk * 512 + cw)
                        for hs in range(NH):
                            p1t = pp1.tile([P, cw], dt.float32, tag="p1")
                            for ds in range(ND):
                                nc.tensor.matmul(
                                    out=p1t[:],
                                    lhsT=w1sb[:, ds, hs * P:(hs + 1) * P],
                                    rhs=xg[:, ds, ks],
                                    start=(ds == 0), stop=(ds == ND - 1))
                            nc.scalar.activation(
                                out=hsb[:, hs, ks], in_=p1t[:], func=AF.Relu,
                                bias=b1sb[:, e * NH + hs:e * NH + hs + 1])

                    # prefetch next expert's weights
                    if e + 1 < E:
                        w1n = wtp.tile([P, ND, H], dt.float16, tag="w1")
                        w2n = wtp.tile([P, NH, O], dt.float16, tag="w2")
                        for ds in range(ND):
                            nc.sync.dma_start(
                                w1n[:, ds, :],
                                bass.AP(tensor=w1[:, :, :].tensor,
                                        offset=(e + 1) * D * H + ds * P * H,
                                        ap=[[H, P], [1, H]]))
                            nc.scalar.dma_start(
                                w2n[:, ds, :],
                                bass.AP(tensor=w2[:, :, :].tensor,
                                        offset=(e + 1) * H * O + ds * P * O,
                                        ap=[[O, P], [1, O]]))

                    # mm2 + gate + scatter-add combine
                    for tt in range(TC[e]):
                        ts_ = slice(tt * P, (tt + 1) * P)
                        ybf = ybp.tile([P, 1, O], dt.bfloat16, tag="ybf")
                        for oc in range(2):
                            os_ = slice(oc * 512, (oc + 1) * 512)
                            p2t = pp2.tile([P, 512], dt.float32, tag="p2")
                            for hs in range(NH):
                                nc.tensor.matmul(
                                    out=p2t[:], lhsT=hsb[:, hs, ts_],
                                    rhs=w2sb[:, hs, os_],
                                    start=(hs == 0), stop=(hs == NH - 1))
                            nc.vector.tensor_scalar(
                                ybf[:, 0, os_], p2t[:], gatT[:, tt:tt + 1],
                                None, op0=ALU.mult)
                        nc.gpsimd.dma_scatter_add(
                            ysum[:, :], ybf[:], idxr[:, 8 * tt:8 * (tt + 1)],
                            P, P, O)
                    if e + 1 < E:
                        w1sb, w2sb = w1n, w2n

    nc.compile()
    _NC_CACHE[key] = nc
    return nc


def _route_and_assign(xf, Wr, br):
    """Host routing (fp32, matches reference bitwise) + balanced core assign.

    Returns perm [NCORES, T] token ids per core and per-expert capacities.
    """
    logits = xf @ Wr + br
    a = np.argsort(-logits, axis=1)[:, :2].astype(np.int32)
    # greedy balanced assignment, vectorized in chunks for speed
    counts = np.zeros((NCORES, E), np.int32)
    fill = np.zeros(NCORES, np.int32)
    assign = np.empty(TOK, np.int32)
    for t in range(TOK):
        e1, e2 = a[t, 0], a[t, 1]
        load = np.maximum(counts[:, e1], counts[:, e2]).astype(np.float64)
        load[fill >= T] = np.inf
        load += fill * 1e-4
        c = int(np.argmin(load))
        assign[t] = c
        counts[c, e1] += 1
        counts[c, e2] += 1
        fill[c] += 1
    perm = np.empty((NCORES, T), np.int64)
    for c in range(NCORES):
        ids = np.where(assign == c)[0]
        assert len(ids) == T, (c, len(ids))
        perm[c] = ids
    maxc = counts.max(axis=0)
    caps = tuple(int(-(-(m + 16) // P) * P) for m in maxc)
    assert all(cp <= 640 for cp in caps), caps  # gscr stride/SBUF sizing
    return perm, caps


def prep_host(x, Wr, br, W1, b1, W2, b2):
    x = np.ascontiguousarray(np.asarray(x, dtype=np.float32))
    Wr = np.ascontiguousarray(np.asarray(Wr, dtype=np.float32))
    br = np.asarray(br, dtype=np.float32)
    W1 = np.ascontiguousarray(np.asarray(W1, dtype=np.float32))
    b1 = np.asarray(b1, dtype=np.float32)
    W2 = np.ascontiguousarray(np.asarray(W2, dtype=np.float32))
    b2 = np.ascontiguousarray(np.asarray(b2, dtype=np.float32))

    xf = x.reshape(TOK, D)
    perm, caps = _route_and_assign(xf, Wr, br)

    xf16 = xf.astype(np.float16)
    xres = ((xf - xf16.astype(np.float32)) * RS).astype(np.float16)
    wr16 = Wr.astype(np.float16)
    wres = ((Wr - wr16.astype(np.float32)) * RS).astype(np.float16)
    wrcat = np.ascontiguousarray(np.concatenate([wr16, wres], axis=1))
    brc = np.ascontiguousarray(br.reshape(E, 1))
    b1c = np.ascontiguousarray(
        b1.reshape(E, NH, P).transpose(2, 0, 1).reshape(P, E * NH))
    w1f = W1.astype(np.float16)
    w2f = W2.astype(np.float16)
    pp, ff = np.meshgrid(np.arange(16), np.arange(P), indexing="ij")
    iop1 = np.tile((ff * 16 + pp + 1).astype(np.float32), (1, E))
    pw, fw = np.meshgrid(np.arange(16), np.arange(40), indexing="ij")
    wiot = (fw * 16 + pw).astype(np.float32)

    in_maps = []
    for c in range(NCORES):
        ids = perm[c]
        xr = np.ascontiguousarray(xf16[ids])
        in_maps.append({
            "xf16T": np.ascontiguousarray(xr.T),
            "xresT": np.ascontiguousarray(xres[ids].T),
            "xrow": xr,
            "wrcat": wrcat, "brc": brc, "w1": w1f, "b1c": b1c,
            "w2": w2f, "b2": b2, "iop1": iop1, "wiot": wiot,
        })
    return in_maps, perm, caps


def kernel(x, Wr, br, W1, b1, W2, b2):
    in_maps, perm, caps = prep_host(x, Wr, br, W1, b1, W2, b2)
    nc = build_nc(caps)
    res = run_bass_kernel_spmd(nc, in_maps, core_ids=list(range(NCORES)))
    out = np.empty((TOK, O), np.float32)
    for c in range(NCORES):
        out[perm[c]] = res.results[c]["ysum"].astype(np.float32)
    return out.reshape(B, NOBJ, O)


# revision 37
# speedup vs baseline: 4.7868x; 2.1336x over previous
"""Sparse top-2 MoE kernel for Trainium2, 8 NeuronCores, data-parallel tokens.

Problem: x (4, 4096, 1024), router Wr (1024, 8) + br, experts W1 (8,1024,1024)
+ b1, W2 (8,1024,1024) + b2, top-2 softmax routing, out (4, 4096, 1024).

Strategy (vs dense baseline that computed all 8 experts per token):
- 16384 tokens split 8 ways (2048/core) with a host-side balanced shuffle so
  per-(core,expert) routed counts stay near the mean; expert weights
  replicated in fp16.
- Router logits computed fp32-exact via fp16 hi/lo split (x = x16 + 2^-11*xr,
  Wr likewise), so top-2 selection matches the fp32 reference bit-for-bit
  (min top2/top3 logit gap 6.5e-6 >> 2e-6 reconstruction error).
- On-device compaction: per expert, sparse_gather compacts the selected
  token ids + gate coefs out of a wrapped-16 mask layout; static per-expert
  capacities (actual count + 16 margin, rounded to 128) keep the NEFF static.
- dma_gather (transposed) fetches only the routed tokens' x rows in fp16;
  mm1/mm2 run fp16 at full PE rate on ~2*T tokens instead of 8*T.
- Combine: y tiles scaled by gate coef on evac, dma_scatter_add'ed into the
  bf16 output accumulator (pre-initialized with the coef@b2 bias term).
Host adds nothing: out = per-core ysum rows un-permuted, upcast to f32.
"""
import sys

sys.path.insert(0, "/opt/trn_rl_repo")

import numpy as np
import ml_dtypes
import concourse.bass as bass
import concourse.mybir as mybir
import concourse.tile as tile
from concourse import bacc, library_config
from concourse.bass_utils import run_bass_kernel_spmd
from concourse.masks import make_identity

dt = mybir.dt
AF = mybir.ActivationFunctionType
ALU = mybir.AluOpType

NCORES = 8
B, NOBJ, D = 4, 4096, 1024
H = O = 1024
E = 8
TOK = B * NOBJ          # 16384 tokens total
T = TOK // NCORES       # 2048 tokens per core
P = 128
ND = D // P             # 8 d-slices
NH = H // P             # 8 h-slices
NTT = T // P            # 16 token tiles
RS = 2048.0             # hi/lo residual scale 2^11

_NC_CACHE = {}


def build_nc(caps, body_reps=1, dbg=False):
    """caps: tuple of 8 per-expert capacities (multiples of 128)."""
    key = (tuple(caps), body_reps, dbg)
    if key in _NC_CACHE:
        return _NC_CACHE[key]
    nc = bacc.Bacc("TRN2", target_bir_lowering=False, debug=False)

    xcatT = nc.dram_tensor("xcatT", [D, 2, T], dt.float16, kind="ExternalInput")
    xrow = nc.dram_tensor("xrow", [T, D], dt.float16, kind="ExternalInput")
    wrcat = nc.dram_tensor("wrcat", [D, 40], dt.float16, kind="ExternalInput")
    brc = nc.dram_tensor("brc", [40, 1], dt.float32, kind="ExternalInput")
    w1 = nc.dram_tensor("w1", [E, D, H], dt.float16, kind="ExternalInput")
    b1c = nc.dram_tensor("b1c", [P, E * NH], dt.float32, kind="ExternalInput")
    w2 = nc.dram_tensor("w2", [E, H, O], dt.float16, kind="ExternalInput")
    b2 = nc.dram_tensor("b2", [E, O], dt.float32, kind="ExternalInput")
    iop1 = nc.dram_tensor("iop1", [16, E * P], dt.float32, kind="ExternalInput")
    wiot = nc.dram_tensor("wiot", [16, 40], dt.float32, kind="ExternalInput")
    scrm = nc.dram_tensor("scrm", [3 * T], dt.float32, kind="Internal")
    lgscr = nc.dram_tensor("lgscr", [E * T], dt.float32, kind="Internal")
    cfscr = nc.dram_tensor("cfscr", [E * T], dt.float32, kind="Internal")
    gscr = nc.dram_tensor("gscr", [E * 640], dt.float32, kind="Internal")
    nfscr = nc.dram_tensor("nfscr", [E * 16], dt.float32, kind="Internal")
    ysum = nc.dram_tensor("ysum", [T, O], dt.bfloat16, kind="ExternalOutput")
    if dbg:
        CMX = max(caps)
        dlog = nc.dram_tensor("dlog", [E, T], dt.float32, kind="ExternalOutput")
        dsti = nc.dram_tensor("dsti", [16, E * P], dt.float32, kind="ExternalOutput")
        dsci = nc.dram_tensor("dsci", [16, E * P], dt.float32, kind="ExternalOutput")
        dtok = nc.dram_tensor("dtok", [16, E * (CMX // 16)], dt.float32, kind="ExternalOutput")
        dcfw = nc.dram_tensor("dcfw", [16, E * (CMX // 16)], dt.float32, kind="ExternalOutput")
        dgat = nc.dram_tensor("dgat", [P, E * (CMX // P)], dt.float32, kind="ExternalOutput")
        didx = nc.dram_tensor("didx", [P, E * (CMX // 16)], dt.int16, kind="ExternalOutput")

    CT = [c // 16 for c in caps]    # wrapped free width per expert
    TC = [c // P for c in caps]     # token tiles per expert

    with tile.TileContext(nc) as tc:
        with (
            tc.tile_pool(name="const", bufs=1) as cpool,
            tc.tile_pool(name="rx", bufs=1) as rxp,
            tc.tile_pool(name="lgt", bufs=1) as lgp,
            tc.tile_pool(name="topk", bufs=2) as tkp,
            tc.tile_pool(name="mask", bufs=1) as mkp,
            tc.tile_pool(name="sg", bufs=1) as sgp,
            tc.tile_pool(name="idx", bufs=3) as idxp,
            tc.tile_pool(name="wt", bufs=2) as wtp,
            tc.tile_pool(name="xg", bufs=2) as xgp,
            tc.tile_pool(name="hb", bufs=2) as hbp,
            tc.tile_pool(name="yb", bufs=3) as ybp,
            tc.tile_pool(name="pr", bufs=2, space="PSUM") as ppr,
            tc.tile_pool(name="p1", bufs=2, space="PSUM") as pp1,
            tc.tile_pool(name="p2", bufs=2, space="PSUM") as pp2,
        ):
            nc.gpsimd.load_library(library_config.sparse_gather)

            ident = cpool.tile([P, P], dt.float32)
            wrsb = cpool.tile([P, ND * 40], dt.float16)
            nc.sync.dma_start(
                wrsb[:], bass.AP(tensor=wrcat[:, :].tensor, offset=0,
                                 ap=[[40, P], [P * 40, ND], [1, 40]]))
            brsb = cpool.tile([40, 1], dt.float32)
            b1sb = cpool.tile([P, E * NH], dt.float32)
            b2sb = cpool.tile([E, O], dt.float32r)
            iosb = cpool.tile([16, E * P], dt.float32)
            wiosb = cpool.tile([16, 40], dt.float32)

            for rep in range(body_reps):
                # ---- router: logitsT [8, 2048] fp32-exact via hi/lo fp16
                # (x-tile DMAs emitted first so the PE isn't gated behind
                # the weight prefetch on the SP queue)
                logitsT = lgp.tile([E, T], dt.float32, tag="lgt")
                xtiles = []
                for ch in range(4):
                    row = []
                    for ds in range(ND):
                        xc = rxp.tile([P, 2, 512], dt.float16, tag="xc",
                                      bufs=16)
                        if ch < 2:
                            qe = nc.sync if ds % 2 == 0 else nc.scalar
                        else:
                            qe = nc.gpsimd
                        qe.dma_start(
                            xc[:],
                            bass.AP(tensor=xcatT[:, :, :].tensor,
                                    offset=ds * P * 2 * T + ch * 512,
                                    ap=[[2 * T, P], [T, 2], [1, 512]]))
                        row.append(xc)
                    xtiles.append(row)

                if rep == 0:
                    nc.sync.dma_start(brsb[:], brc[:])
                    make_identity(nc, ident[:])
                    nc.scalar.dma_start(b1sb[:], b1c[:])
                    nc.scalar.dma_start(b2sb[:], b2[:].bitcast(dt.float32r))
                    nc.scalar.dma_start(iosb[:], iop1[:])
                    nc.scalar.dma_start(wiosb[:], wiot[:])
                # ---- w1/w2 prefetch for expert 0 (off the SP queue)
                w1sb = wtp.tile([P, ND, H], dt.float16, tag="w1")
                w2sb = wtp.tile([P, NH, O], dt.float16, tag="w2")
                for ds in range(ND):
                    nc.gpsimd.dma_start(
                        w1sb[:, ds, :],
                        bass.AP(tensor=w1[:, :, :].tensor, offset=ds * P * H,
                                ap=[[H, P], [1, H]]))
                    nc.scalar.dma_start(
                        w2sb[:, ds, :],
                        bass.AP(tensor=w2[:, :, :].tensor, offset=ds * P * O,
                                ap=[[O, P], [1, O]]))

                for ch in range(4):
                    cs = slice(ch * 512, (ch + 1) * 512)
                    pA = ppr.tile([40, 512], dt.float32, tag="pA")
                    pC = ppr.tile([E, 512], dt.float32, tag="pC", bufs=1)
                    for ds in range(ND):
                        xc = xtiles[ch][ds]
                        nc.tensor.matmul(
                            out=pA[:], lhsT=wrsb[:, 40 * ds:40 * (ds + 1)],
                            rhs=xc[:, 0, :], start=(ds == 0),
                            stop=(ds == ND - 1))
                        nc.tensor.matmul(
                            out=pC[:], lhsT=wrsb[:, 40 * ds:40 * ds + 8],
                            rhs=xc[:, 1, :], start=(ds == 0),
                            stop=(ds == ND - 1))
                    # evac: wr16.T x16 rows 0:8, wres.T x16 rows 32:40 (the
                    # 24-col zero pad keeps wres at a 32-aligned partition);
                    # bias rides on the hi rows (brsb rows 8+ are zero)
                    sb16 = tkp.tile([40, 512], dt.float32, tag="sb16")
                    nc.vector.tensor_scalar(
                        sb16[:], pA[:], brsb[:, 0:1], None, op0=ALU.add)
                    t1 = tkp.tile([E, 512], dt.float32, tag="t1")
                    nc.vector.scalar_tensor_tensor(
                        out=t1[:], in0=pC[:], scalar=0.0, in1=sb16[32:40, :],
                        op0=ALU.bypass, op1=ALU.add)
                    nc.vector.scalar_tensor_tensor(
                        out=logitsT[:, cs], in0=t1[:], scalar=1.0 / RS,
                        in1=sb16[0:8, :], op0=ALU.mult, op1=ALU.add)
                    nc.sync.dma_start(
                        bass.AP(tensor=lgscr[:].tensor, offset=ch * 512,
                                ap=[[T, E], [1, 512]]),
                        logitsT[:, cs])

                # ---- top2, grouped: one transposed [128, 16*8] block
                m12c = tkp.tile([P, 48], dt.float32, tag="m12c")
                plA = pp1.tile([P, NTT * E], dt.float32, tag="pst", bufs=1)
                for tt in range(NTT):
                    nc.tensor.matmul(
                        out=plA[:, E * tt:E * (tt + 1)],
                        lhsT=logitsT[:, tt * P:(tt + 1) * P],
                        rhs=ident[:E, :E], is_transpose=True,
                        start=(tt == 0), stop=(tt == NTT - 1),
                        skip_group_check=True)
                lgall = tkp.tile([P, NTT * E], dt.float32, tag="lgall")
                nc.scalar.copy(lgall[:], plA[:])
                lg3 = lgall[:].rearrange("p (t e) -> p t e", e=E)
                nc.vector.tensor_reduce(m12c[:, 0:16], lg3,
                                        axis=mybir.AxisListType.X,
                                        op=ALU.max)
                eqm = tkp.tile([P, NTT * E], dt.float32, tag="eqm")
                eq3 = eqm[:].rearrange("p (t e) -> p t e", e=E)
                nc.vector.scalar_tensor_tensor(
                    out=eq3, in0=lg3, scalar=0.0,
                    in1=m12c[:, 0:16].unsqueeze(2).broadcast_to([P, NTT, E]),
                    op0=ALU.bypass, op1=ALU.is_equal)
                nc.vector.scalar_tensor_tensor(
                    out=eq3, in0=eq3, scalar=-1e30, in1=lg3,
                    op0=ALU.mult, op1=ALU.add)
                nc.vector.tensor_reduce(m12c[:, 16:32], eq3,
                                        axis=mybir.AxisListType.X,
                                        op=ALU.max)
                d01a = tkp.tile([P, 16], dt.float32, tag="d01a")
                nc.vector.scalar_tensor_tensor(
                    out=d01a[:], in0=m12c[:, 16:32], scalar=-1.0,
                    in1=m12c[:, 0:16], op0=ALU.mult, op1=ALU.add)
                nc.scalar.activation(out=m12c[:, 32:48], in_=d01a[:],
                                     func=AF.Sigmoid)

                if dbg:
                    nc.sync.dma_start(dlog[:], logitsT[:])
                # ---- to wrapped-16 layout via DRAM roundtrip
                nc.scalar.dma_start(
                    bass.AP(tensor=scrm[:].tensor, offset=0,
                            ap=[[1, P], [T, 3], [P, 16]]),
                    m12c[:])
                # layout [16, E*128]: expert e in free cols [128e, 128e+128),
                # wrapped-16 within (gpsimd ISA ops need partition base 0).
                # m1/m2/c0 stay [16,128] and broadcast along the expert dim.
                mt = [mkp.tile([16, E * P], dt.float32, name=f"mt{i}")
                      for i in range(5)]
                lgW, eq1, eq2, cfE, tmpE = mt
                m1S = mkp.tile([16, P], dt.float32, name="m1S")
                m2S = mkp.tile([16, P], dt.float32, name="m2S")
                c0S = mkp.tile([16, P], dt.float32, name="c0S")
                c1S = mkp.tile([16, P], dt.float32, name="c1S")
                for g in range(8):
                    gs = slice(P * g, P * (g + 1))
                    nc.sync.dma_start(
                        lgW[:, gs],
                        bass.AP(tensor=lgscr[:].tensor, offset=g * T,
                                ap=[[1, 16], [16, P]]))
                nc.scalar.dma_start(
                    m1S[:], bass.AP(tensor=scrm[:].tensor, offset=0,
                                    ap=[[1, 16], [16, P]]))
                nc.scalar.dma_start(
                    m2S[:], bass.AP(tensor=scrm[:].tensor, offset=T,
                                    ap=[[1, 16], [16, P]]))
                nc.scalar.dma_start(
                    c0S[:], bass.AP(tensor=scrm[:].tensor, offset=2 * T,
                                    ap=[[1, 16], [16, P]]))
                nc.vector.tensor_scalar(c1S[:], c0S[:], -1.0, 1.0,
                                        op0=ALU.mult, op1=ALU.add)


                # ---- masks and compaction inputs  [128,128]
                def tt_op(out, a, b, op):
                    nc.vector.scalar_tensor_tensor(
                        out=out, in0=a, scalar=0.0, in1=b,
                        op0=ALU.bypass, op1=op)

                # per-expert mask slices -> compaction, e0 first so the
                # expert pipeline starts while later experts still compact
                CM = max(caps)
                CM16 = CM // 16
                toks, cfs, nfs, idxrs, tmasks = [], [], [], [], []
                for e in range(E):
                    es = slice(P * e, P * (e + 1))
                    nc.vector.scalar_tensor_tensor(
                        out=eq1[:, es], in0=lgW[:, es], scalar=0.0,
                        in1=m1S[:], op0=ALU.bypass, op1=ALU.is_equal)
                    nc.vector.scalar_tensor_tensor(
                        out=eq2[:, es], in0=lgW[:, es], scalar=0.0,
                        in1=m2S[:], op0=ALU.bypass, op1=ALU.is_equal)
                    # cfE = eq1*c0 + eq2*c1 (coef_full, kept for b2 pass)
                    nc.vector.scalar_tensor_tensor(
                        out=cfE[:, es], in0=eq1[:, es], scalar=0.0,
                        in1=c0S[:], op0=ALU.bypass, op1=ALU.mult)
                    nc.vector.scalar_tensor_tensor(
                        out=tmpE[:, es], in0=eq2[:, es], scalar=0.0,
                        in1=c1S[:], op0=ALU.bypass, op1=ALU.mult)
                    tt_op(cfE[:, es], cfE[:, es], tmpE[:, es], ALU.add)
                    # selm2 = 2*(eq1+eq2) - 1
                    tt_op(eq1[:, es], eq1[:, es], eq2[:, es], ALU.add)
                    nc.vector.tensor_scalar(eq1[:, es], eq1[:, es], 2.0,
                                            -1.0, op0=ALU.mult, op1=ALU.add)
                    # sci = (cfE+1)*selm2 ; sti = (tau+1)*selm2
                    nc.vector.scalar_tensor_tensor(
                        out=tmpE[:, es], in0=cfE[:, es], scalar=1.0,
                        in1=eq1[:, es], op0=ALU.add, op1=ALU.mult)
                    tt_op(eq2[:, es], iosb[:, es], eq1[:, es], ALU.mult)
                    # compact this expert now (Pool runs these in order)
                    tokw = sgp.tile([16, CT[e]], dt.float32, name=f"tok{e}")
                    cfw = sgp.tile([16, CT[e]], dt.float32, name=f"cf{e}")
                    nf1 = sgp.tile([1, 1], dt.uint32, name=f"nf1{e}")
                    nf2 = sgp.tile([1, 1], dt.uint32, name=f"nf2{e}")
                    nc.gpsimd.sparse_gather(tokw[:], eq2[:, es],
                                            num_found=nf1[:])
                    nc.gpsimd.sparse_gather(cfw[:], tmpE[:, es],
                                            num_found=nf2[:])
                    toks.append(tokw)
                    cfs.append(cfw)
                    nfs.append(nf1)
                    # spill coef_full column block for the b2 pass
                    nc.sync.dma_start(
                        bass.AP(tensor=cfscr[:].tensor, offset=e * T,
                                ap=[[1, 16], [16, P]]),
                        cfE[:, P * e:P * (e + 1)])
                    # nf -> [16,1] via free-broadcast + DRAM fold (no gpsimd)
                    nfw = idxp.tile([1, 16], dt.float32, tag="nfw")
                    nc.vector.tensor_copy(
                        nfw[:], nfs[e][:].broadcast_to([1, 16]))
                    nc.scalar.dma_start(
                        bass.AP(tensor=nfscr[:].tensor, offset=e * 16,
                                ap=[[1, 1], [1, 16]]),
                        nfw[:])
                    nfT = idxp.tile([16, 1], dt.float32, tag="nfT")
                    nc.scalar.dma_start(
                        nfT[:], bass.AP(tensor=nfscr[:].tensor,
                                        offset=e * 16, ap=[[1, 16], [1, 1]]))
                    tmask = sgp.tile([16, CT[e]], dt.float32,
                                     name=f"tmask{e}")
                    nc.vector.tensor_scalar(tmask[:], wiosb[:, :CT[e]],
                                            nfT[:, 0:1], None, op0=ALU.is_lt)
                    tmasks.append(tmask)
                    # idx list: tau+1 -> tau, tail -> 0, int16, replicate
                    tokc = idxp.tile([16, CM16], dt.float32, tag="tokc")
                    nc.vector.memset(tokc[:], 0.0)
                    nc.vector.tensor_scalar(tokc[:, :CT[e]], tokw[:],
                                            -1.0, 0.0, op0=ALU.add,
                                            op1=ALU.max)
                    tt_op(tokc[:, :CT[e]], tokc[:, :CT[e]], tmask[:],
                          ALU.mult)
                    idxr = sgp.tile([P, CM16], dt.int16, name=f"idxr{e}")
                    nc.vector.tensor_copy(idxr[0:16, :], tokc[:])
                    nc.sync.dma_start(idxr[16:32, :], idxr[0:16, :])
                    nc.sync.dma_start(idxr[32:64, :], idxr[0:32, :])
                    nc.sync.dma_start(idxr[64:128, :], idxr[0:64, :])
                    idxrs.append(idxr)
                sti = eq2
                sci = tmpE
                if dbg:
                    nc.sync.dma_start(dsti[:], sti[:])
                    nc.sync.dma_start(dsci[:], sci[:])
                coefT = lgp.tile([E, T], dt.float32r, tag="lgt")
                nc.scalar.dma_start(coefT[:], cfscr[:].bitcast(dt.float32r))
                if dbg:
                    for e in range(E):
                        nc.sync.dma_start(
                            dtok[:, e * (CMX // 16):e * (CMX // 16) + CT[e]],
                            toks[e][:])
                        nc.sync.dma_start(
                            dcfw[:, e * (CMX // 16):e * (CMX // 16) + CT[e]],
                            cfs[e][:])
                nc.gpsimd.load_library(library_config.mlp)

                # ---- b2 bias pass: ysum = coefT.T @ b2 (bf16 init)
                for tt in range(NTT):
                    ts_ = slice(tt * P, (tt + 1) * P)
                    yb = ybp.tile([P, 1, O], dt.bfloat16, tag="ybias")
                    for oc in range(2):
                        os_ = slice(oc * 512, (oc + 1) * 512)
                        pb = pp2.tile([P, 512], dt.float32, tag="p2")
                        nc.tensor.matmul(out=pb[:], lhsT=coefT[:, ts_],
                                         rhs=b2sb[:, os_], start=True,
                                         stop=True)
                        nc.vector.tensor_copy(yb[:, 0, os_], pb[:])
                    nc.scalar.dma_start(ysum[ts_, :], yb[:, 0, :])

                # ---- experts
                for e in range(E):
                    C = caps[e]
                    idxr = idxrs[e]
                    # gating (coef+1 -> coef, 0 in padded tail); off the
                    # pre-switch critical path -- mm2 needs it much later
                    gatw = idxp.tile([16, CM16], dt.float32, tag="gatw")
                    nc.vector.memset(gatw[:], 0.0)
                    nc.vector.tensor_scalar(gatw[:, :CT[e]], cfs[e][:],
                                            -1.0, 0.0, op0=ALU.add,
                                            op1=ALU.max)
                    tt_op(gatw[:, :CT[e]], gatw[:, :CT[e]], tmasks[e][:],
                          ALU.mult)
                    nc.scalar.dma_start(
                        bass.AP(tensor=gscr[:].tensor, offset=e * 640,
                                ap=[[1, 16], [16, CT[e]]]),
                        gatw[:, :CT[e]])
                    gatT = idxp.tile([P, CM // P], dt.float32, tag="gatT")
                    nc.scalar.dma_start(
                        gatT[:, :TC[e]],
                        bass.AP(tensor=gscr[:].tensor, offset=e * 640,
                                ap=[[1, P], [P, TC[e]]]))
                    if dbg:
                        nc.sync.dma_start(
                            dgat[:, e * (CMX // P):e * (CMX // P) + TC[e]],
                            gatT[:, :TC[e]])
                        nc.sync.dma_start(
                            didx[:, e * (CMX // 16):e * (CMX // 16) + CT[e]],
                            idxr[:, :CT[e]])
                    # dispatch: gather routed tokens' x rows (transposed)
                    xg = xgp.tile([P, ND, CM], dt.float16, tag="xg")
                    nc.gpsimd.dma_gather(xg[:], xrow[:, :], idxr[:], CM, CM,
                                         D, transpose=True)

                    # mm1: h = relu(x @ W1[e] + b1[e]), fp16
                    hsb = hbp.tile([P, NH, CM], dt.float16, tag="h")
                    nch = (C + 511) // 512
                    for chk in range(nch):
                        cw = min(512, C - chk * 512)
                        ks = slice(chk * 512, chk * 512 + cw)
                        for hs in range(NH):
                            p1t = pp1.tile([P, cw], dt.float32, tag="p1")
                            for ds in range(ND):
                                nc.tensor.matmul(
                                    out=p1t[:],
                                    lhsT=w1sb[:, ds, hs * P:(hs + 1) * P],
                                    rhs=xg[:, ds, ks],
                                    start=(ds == 0), stop=(ds == ND - 1))
                            nc.scalar.activation(
                                out=hsb[:, hs, ks], in_=p1t[:], func=AF.Relu,
                                bias=b1sb[:, e * NH + hs:e * NH + hs + 1])

                    # prefetch next expert's weights
                    if e + 1 < E:
                        w1n = wtp.tile([P, ND, H], dt.float16, tag="w1")
                        w2n = wtp.tile([P, NH, O], dt.float16, tag="w2")
                        for ds in range(ND):
                            nc.sync.dma_start(
                                w1n[:, ds, :],
                                bass.AP(tensor=w1[:, :, :].tensor,
                                        offset=(e + 1) * D * H + ds * P * H,
                                        ap=[[H, P], [1, H]]))
                            nc.scalar.dma_start(
                                w2n[:, ds, :],
                                bass.AP(tensor=w2[:, :, :].tensor,
                                        offset=(e + 1) * H * O + ds * P * O,
                                        ap=[[O, P], [1, O]]))

                    # mm2 + gate + scatter-add combine
                    for tt in range(TC[e]):
                        ts_ = slice(tt * P, (tt + 1) * P)
                        ybf = ybp.tile([P, 1, O], dt.bfloat16, tag="ybf")
                        for oc in range(2):
                            os_ = slice(oc * 512, (oc + 1) * 512)
                            p2t = pp2.tile([P, 512], dt.float32, tag="p2")
                            for hs in range(NH):
                                nc.tensor.matmul(
                                    out=p2t[:], lhsT=hsb[:, hs, ts_],
                                    rhs=w2sb[:, hs, os_],
                                    start=(hs == 0), stop=(hs == NH - 1))
                            nc.vector.tensor_scalar(
                                ybf[:, 0, os_], p2t[:], gatT[:, tt:tt + 1],
                                None, op0=ALU.mult)
                        nc.gpsimd.dma_scatter_add(
                            ysum[:, :], ybf[:], idxr[:, 8 * tt:8 * (tt + 1)],
                            P, P, O)
                    if e + 1 < E:
                        w1sb, w2sb = w1n, w2n

    nc.compile()
    _NC_CACHE[key] = nc
    return nc


def _route_and_assign(xf, Wr, br):
    """Host routing (fp32, matches reference bitwise) + balanced core assign.

    Returns perm [NCORES, T] token ids per core and per-expert capacities.
    """
    logits = xf @ Wr + br
    a = np.argsort(-logits, axis=1)[:, :2].astype(np.int32)
    # greedy balanced assignment, vectorized in chunks for speed
    counts = np.zeros((NCORES, E), np.int32)
    fill = np.zeros(NCORES, np.int32)
    assign = np.empty(TOK, np.int32)
    for t in range(TOK):
        e1, e2 = a[t, 0], a[t, 1]
        load = np.maximum(counts[:, e1], counts[:, e2]).astype(np.float64)
        load[fill >= T] = np.inf
        load += fill * 1e-4
        c = int(np.argmin(load))
        assign[t] = c
        counts[c, e1] += 1
        counts[c, e2] += 1
        fill[c] += 1
    perm = np.empty((NCORES, T), np.int64)
    for c in range(NCORES):
        ids = np.where(assign == c)[0]
        assert len(ids) == T, (c, len(ids))
        perm[c] = ids
    maxc = counts.max(axis=0)
    caps = tuple(int(-(-(m + 16) // P) * P) for m in maxc)
    assert all(cp <= 640 for cp in caps), caps  # gscr stride/SBUF sizing
    return perm, caps


def prep_host(x, Wr, br, W1, b1, W2, b2):
    x = np.ascontiguousarray(np.asarray(x, dtype=np.float32))
    Wr = np.ascontiguousarray(np.asarray(Wr, dtype=np.float32))
    br = np.asarray(br, dtype=np.float32)
    W1 = np.ascontiguousarray(np.asarray(W1, dtype=np.float32))
    b1 = np.asarray(b1, dtype=np.float32)
    W2 = np.ascontiguousarray(np.asarray(W2, dtype=np.float32))
    b2 = np.ascontiguousarray(np.asarray(b2, dtype=np.float32))

    xf = x.reshape(TOK, D)
    perm, caps = _route_and_assign(xf, Wr, br)

    xf16 = xf.astype(np.float16)
    xres = ((xf - xf16.astype(np.float32)) * RS).astype(np.float16)
    wr16 = Wr.astype(np.float16)
    wres = ((Wr - wr16.astype(np.float32)) * RS).astype(np.float16)
    wrcat = np.zeros((D, 40), np.float16)
    wrcat[:, 0:8] = wr16
    wrcat[:, 32:40] = wres
    wrcat = np.ascontiguousarray(wrcat)
    brc = np.ascontiguousarray(
        np.concatenate([br, np.zeros(32, np.float32)]).reshape(40, 1))
    b1c = np.ascontiguousarray(
        b1.reshape(E, NH, P).transpose(2, 0, 1).reshape(P, E * NH))
    w1f = W1.astype(np.float16)
    w2f = W2.astype(np.float16)
    pp, ff = np.meshgrid(np.arange(16), np.arange(P), indexing="ij")
    iop1 = np.tile((ff * 16 + pp + 1).astype(np.float32), (1, E))
    pw, fw = np.meshgrid(np.arange(16), np.arange(40), indexing="ij")
    wiot = (fw * 16 + pw).astype(np.float32)

    in_maps = []
    for c in range(NCORES):
        ids = perm[c]
        xr = np.ascontiguousarray(xf16[ids])
        in_maps.append({
            "xcatT": np.ascontiguousarray(
                np.stack([xr.T, xres[ids].T], axis=1)),
            "xrow": xr,
            "wrcat": wrcat, "brc": brc, "w1": w1f, "b1c": b1c,
            "w2": w2f, "b2": b2, "iop1": iop1, "wiot": wiot,
        })
    return in_maps, perm, caps


def kernel(x, Wr, br, W1, b1, W2, b2):
    in_maps, perm, caps = prep_host(x, Wr, br, W1, b1, W2, b2)
    nc = build_nc(caps)
    res = run_bass_kernel_spmd(nc, in_maps, core_ids=list(range(NCORES)))
    out = np.empty((TOK, O), np.float32)
    for c in range(NCORES):
        out[perm[c]] = res.results[c]["ysum"].astype(np.float32)
    return out.reshape(B, NOBJ, O)
